# revision 60
# baseline (speedup 1.0000x reference)
"""MultiHeadedAttention Trainium2 kernel.

Problem: B=2, S=4096, d_model=512, H=8 heads, dk=64.
  q/k/v proj -> scaled dot-product attention per head -> concat -> out proj.

Sharding: 8 cores = (batch b in {0,1}) x (head-pair hp in {0..3}).
Each core computes, for its batch and its 2 heads:
  QpT/KpT/VpT = (x @ W[:, hp*128:hp*128+128] + b).T          [128=2*dk, 4096]
  S^T[k, q]   = Kp Qp^T (per head, fp32r matmuls, k-major)
  P^T         = exp(S^T / 8)  (ScalarE, reads PSUM, writes SBUF)
  ctxT, Z     = [Vp | 1].T @ P^T  (ones column gives softmax denominator)
  ctxT       /= Z  (reciprocal + partition_broadcast + multiply)
  outT_partial= Wo[hp*128:...]^T-chunks @ ctx   -> [512, 4096] (transposed)
Host: out[b] = sum_hp(outT_partial).T + bo.

Inputs are fed pre-transposed ([d_model, S], c-major) and host-converted to
bf16 (x and Wq/Wk/Wv) to halve input DMA; projection matmuls contract over
the partition dimension with no on-device transposes of x.  The serial
projection phase is split into DMA kicks + matmul steps streamed through
q-tile 0's kb loop so the Scalar engine's exp starts ~2us in; outputs are
written back as bf16 partials summed on host.
"""

import ml_dtypes
import numpy as np

import concourse.bass as bass
import concourse.bacc as bacc
import concourse.mybir as mybir
import concourse.tile as tile
from concourse.bass_utils import run_bass_kernel_spmd
from concourse.masks import make_identity

F32 = mybir.dt.float32
F32R = mybir.dt.float32r
BF16 = mybir.dt.bfloat16
EXP = mybir.ActivationFunctionType.Exp

B = 2
S = 4096
D = 512           # d_model
H = 8
DK = 64
HP = 4            # head pairs per batch
DL = 128          # local channels per core (2 heads)
CJ = 4            # contraction chunks of 128 over d_model
QT = S // 512     # 8 q-tiles of 512
KB = S // 128     # 32 k-blocks of 128
SCALE = 1.0 / np.sqrt(DK).item()  # 1/8
CTX_LAG = 4

TRACE = False
LAST_RESULTS = None

_BF = ml_dtypes.bfloat16

_prog_cache = {}


def _emit(nc, reps=1):
    xqT = nc.declare_dram_parameter("xqT", [D, S], BF16, isOutput=False)
    xkT = nc.declare_dram_parameter("xkT", [D, S], BF16, isOutput=False)
    xvT = nc.declare_dram_parameter("xvT", [D, S], BF16, isOutput=False)
    wq = nc.declare_dram_parameter("wq", [D, DL], BF16, isOutput=False)
    wk = nc.declare_dram_parameter("wk", [D, DL], BF16, isOutput=False)
    wv = nc.declare_dram_parameter("wv", [D, DL], BF16, isOutput=False)
    bq = nc.declare_dram_parameter("bq", [DL, 1], F32, isOutput=False)
    bk = nc.declare_dram_parameter("bk", [DL, 1], F32, isOutput=False)
    bv = nc.declare_dram_parameter("bv", [DL, 1], F32, isOutput=False)
    wo = nc.declare_dram_parameter("wo", [DL, D], F32, isOutput=False)
    outT = nc.declare_dram_parameter("outT", [D, S], BF16, isOutput=True)

    with tile.TileContext(nc) as tc:
        with (
            nc.allow_low_precision(reason="fp32r (fp22-mantissa) matmul inputs"),
            tc.tile_pool(name="singles", bufs=1) as singles,
            tc.tile_pool(name="xin", bufs=4) as xin,
            tc.tile_pool(name="proj", bufs=1) as proj,
            tc.tile_pool(name="pt", bufs=10) as ptpool,
            tc.tile_pool(name="ctx", bufs=2) as ctxpool,
            tc.tile_pool(name="outp", bufs=4) as outpool,
            tc.tile_pool(name="norm", bufs=6) as normpool,
            tc.tile_pool(name="mm512", bufs=2, space="PSUM") as mmps,
            tc.tile_pool(name="sps", bufs=2, space="PSUM") as spsum,
            tc.tile_pool(name="cps", bufs=2, space="PSUM") as cpsum,
        ):
            # --- constants / weights ---
            ident = singles.tile([128, 128], BF16, tag="ident")
            make_identity(nc, ident)
            ones_stage = singles.tile([128, 64], F32, tag="ones_stage")
            nc.vector.memset(ones_stage, 1.0)
            ones64 = singles.tile([1, 64], F32R, tag="ones64")
            nc.vector.tensor_copy(ones64, ones_stage[0:1, :])
            warm = singles.tile([1, 8], F32, tag="warm")
            nc.scalar.activation(warm, ones_stage[0:1, 0:8], EXP, scale=1.0)

            w_sb = {}
            b_sb = {}

            def load_w(name, w, bias):
                t = singles.tile([128, CJ, DL], BF16, tag=name)
                nc.sync.dma_start(
                    out=t, in_=w[:].rearrange("(j p) d -> p j d", p=128)
                )
                w_sb[name] = t
                bt = singles.tile([DL, 1], F32, tag="b" + name[1])
                nc.sync.dma_start(out=bt, in_=bias[:])
                b_sb["b" + name[1]] = bt

            # --- projections: dst = (x @ W + b).T, channel-major [128, S] ---
            qpT = proj.tile([DL, S], F32R, tag="qpT")
            kpT = proj.tile([DL, S], F32R, tag="kpT")
            vpT = proj.tile([DL, S], BF16, tag="vpT")

            vp = proj.tile([128, KB, 130], BF16, tag="vp")
            nc.vector.tensor_copy(vp[:, :, 64], ones_stage[:, 0:KB])
            nc.vector.tensor_copy(vp[:, :, 129], ones_stage[:, 0:KB])

            xts = {}

            def proj_kick(xT, st, eng=None):
                xTr = xT[:].rearrange("(j p) s -> p j s", p=128)
                xt = xin.tile([128, CJ, 512], BF16, tag="xin")
                (eng or nc.sync).dma_start(
                    out=xt, in_=xTr[:, :, st * 512 : (st + 1) * 512]
                )
                xts[(id(xT), st)] = xt

            def project_st(xT, wname, dst, st):
                """Columns [st*512, ...) of dst = (x @ W + b).T"""
                xt = xts.pop((id(xT), st))
                ps = mmps.tile([128, 512], F32, tag="mm512")
                for cj in range(CJ):
                    nc.tensor.matmul(
                        ps,
                        lhsT=w_sb[wname][:, cj, :],
                        rhs=xt[:, cj, :],
                        start=(cj == 0),
                        stop=(cj == CJ - 1),
                    )
                nc.vector.tensor_scalar_add(
                    dst[:, st * 512 : (st + 1) * 512], ps, b_sb["b" + wname[1]]
                )

            def v_transpose_half(st, half):
                """Vp s-major blocks for 2 of the 4 k-blocks of one s-tile."""
                for kb in range(st * 4 + 2 * half, st * 4 + 2 * half + 2):
                    tp = mmps.tile([128, 512], F32, tag="mm512")
                    tpb = tp[:, 0:64].bitcast(BF16)
                    nc.tensor.transpose(
                        tpb, vpT[:, kb * 128 : (kb + 1) * 128], ident
                    )
                    nc.vector.tensor_copy(vp[:, kb, 0:64], tpb[:, 0:64])
                    nc.vector.tensor_copy(vp[:, kb, 65:129], tpb[:, 64:128])

            # Phase 0 covers only what the first scores/ctx need; the rest
            # of the K/V/Q projections stream through the qt0 kb-loop as
            # per-kb jobs (DMA kicks run a few kb ahead of the matmuls).
            # opening x-kicks go out on the idle Pool SWDGE queue so they
            # do not serialize behind the SP queue's weight-DMA dispatches
            proj_kick(xqT, 0, eng=nc.gpsimd)
            proj_kick(xkT, 0, eng=nc.gpsimd)
            load_w("wq", wq, bq)
            load_w("wk", wk, bk)
            for _w in range(18):
                wtp = mmps.tile([128, 512], F32, tag="mm512")
                nc.tensor.transpose(wtp[:, 0:64].bitcast(BF16), ident, ident)
            project_st(xqT, "wq", qpT, 0)
            load_w("wv", wv, bv)
            proj_kick(xvT, 0)
            project_st(xkT, "wk", kpT, 0)
            proj_kick(xkT, 1)
            wo_sb = singles.tile([DL, D], F32R, tag="wo")
            nc.sync.dma_start(out=wo_sb, in_=wo[:].bitcast(F32R))

            qt0_jobs = {}

            def addjob(kb, fn):
                qt0_jobs.setdefault(kb, []).append(fn)

            addjob(1, (lambda: project_st(xvT, "wv", vpT, 0)))
            addjob(2, (lambda: v_transpose_half(0, 0)))
            addjob(3, (lambda: v_transpose_half(0, 1)))

            for s in range(1, QT):
                addjob(4 * s - 5 if s > 1 else 1,
                       (lambda s=s: proj_kick(xkT, s, eng=nc.gpsimd))
                       if s > 1 else (lambda: None))
                addjob(4 * s - 2, (lambda s=s: project_st(xkT, "wk", kpT, s)))
                addjob(4 * s - 3, (lambda s=s: proj_kick(xvT, s, eng=nc.gpsimd)))
                addjob(4 * s + 1, (lambda s=s: project_st(xvT, "wv", vpT, s)))
                addjob(4 * s + 2, (lambda s=s: v_transpose_half(s, 0)))
                addjob(min(4 * s + 3, KB - 1), (lambda s=s: v_transpose_half(s, 1)))

            # --- attention + output projection, per q-tile of 512 ---
            # Epilogue work (normalize + Wo projection) for q-tile qt is
            # emitted piecewise during q-tile qt+1's kb loop so the PE queue
            # never stalls ACT at the boundary.
            state = {}

            def normalize_h(qt, cps_h, h):
                if h == 0:
                    state["ctx"] = ctxpool.tile([DL, 512], F32R, tag="ctx", name="ctx")
                rec = normpool.tile([1, 512], F32R, tag="rec")
                nc.vector.reciprocal(rec, cps_h[h][64:65, :])
                bc = normpool.tile([64, 512], F32, tag="bc")
                nc.gpsimd.partition_broadcast(bc, rec.bitcast(F32))
                nc.vector.tensor_mul(
                    state["ctx"][h * 64 : (h + 1) * 64, :], cps_h[h][0:64, :], bc
                )

            def oproj_j(qt, j):
                qs = slice(qt * 512, (qt + 1) * 512)
                ops = mmps.tile([128, 512], F32, tag="mm512")
                nc.tensor.matmul(
                    ops,
                    lhsT=wo_sb[:, j * 128 : (j + 1) * 128],
                    rhs=state["ctx"],
                    start=True,
                    stop=True,
                )
                ot = outpool.tile([128, 512], BF16, tag="out")
                nc.vector.tensor_copy(ot, ops)
                nc.sync.dma_start(out=outT[j * 128 : (j + 1) * 128, qs], in_=ot)

            def epilogue_step(step, qt, cps_h):
                if step == 1:
                    normalize_h(qt, cps_h, 0)
                elif step == 2:
                    normalize_h(qt, cps_h, 1)
                elif 3 <= step <= 6:
                    oproj_j(qt, step - 3)

            def scores_exp(qt, kb):
                qs = slice(qt * 512, (qt + 1) * 512)
                sp = spsum.tile([128, 1024], F32, tag="sps")
                for h in (0, 1):
                    nc.tensor.matmul(
                        sp[:, h * 512 : (h + 1) * 512],
                        lhsT=kpT[h * 64 : (h + 1) * 64, kb * 128 : (kb + 1) * 128],
                        rhs=qpT[h * 64 : (h + 1) * 64, qs],
                        start=True,
                        stop=True,
                    )
                pt = ptpool.tile([128, 1024], BF16, tag="pt")
                nc.scalar.activation(pt, sp, EXP, scale=SCALE)
                return pt

            def ctx_mm(cps_h, kb, pt, heads=(0, 1)):
                for h in heads:
                    nc.tensor.matmul(
                        cps_h[h][0:65, :],
                        lhsT=vp[:, kb, 65 * h : 65 * h + 65],
                        rhs=pt[:, h * 512 : (h + 1) * 512],
                        start=(kb == 0),
                        stop=(kb == KB - 1),
                    )

            # Software pipeline: ctx(kb-1) is emitted after scores/exp(kb) so
            # the in-order PE queue never makes ACT wait a full ctx+scores hop.
            pending = None  # (qt, cps_h) awaiting epilogue
            for qt in [q for _ in range(reps) for q in range(QT)]:
                cps0 = cpsum.tile([128, 512], F32, tag="cps")
                cps1 = cpsum.tile([128, 512], F32, tag="cps")
                cps_h = (cps0, cps1)
                pts = {}
                for kb in range(KB):
                    if qt == 0:
                        for fn in qt0_jobs.get(kb, ()):
                            fn()
                    pts[kb] = scores_exp(qt, kb)
                    if kb >= CTX_LAG:
                        ctx_mm(cps_h, kb - CTX_LAG, pts.pop(kb - CTX_LAG))
                    if pending is not None:
                        epilogue_step(kb, *pending)
                    if qt < QT - 1:
                        if kb == 20:
                            proj_kick(xqT, qt + 1)
                        elif kb == 24:
                            project_st(xqT, "wq", qpT, qt + 1)
                if qt == QT - 1:
                    for t in range(KB - CTX_LAG, KB):
                        ctx_mm(cps_h, t, pts[t], heads=(0,))
                    normalize_h(qt, cps_h, 0)
                    for t in range(KB - CTX_LAG, KB):
                        ctx_mm(cps_h, t, pts.pop(t), heads=(1,))
                    normalize_h(qt, cps_h, 1)
                else:
                    for t in range(KB - CTX_LAG, KB):
                        ctx_mm(cps_h, t, pts.pop(t))
                pending = (qt, cps_h)
            for step in range(3, 7):
                epilogue_step(step, *pending)
    return nc


def _build(reps=1):
    if reps not in _prog_cache:
        nc = bacc.Bacc()
        _emit(nc, reps)
        nc.compile()
        _prog_cache[reps] = nc
    return _prog_cache[reps]


def _make_in_maps(query, key, value, Wq, bq, Wk, bk, Wv, bv, Wo):
    in_maps = []
    for b in range(B):
        xqT = np.ascontiguousarray(query[b].T.astype(_BF))
        xkT = np.ascontiguousarray(key[b].T.astype(_BF))
        xvT = np.ascontiguousarray(value[b].T.astype(_BF))
        for hp in range(HP):
            cs = slice(hp * DL, (hp + 1) * DL)
            in_maps.append(
                {
                    "xqT": xqT,
                    "xkT": xkT,
                    "xvT": xvT,
                    "wq": np.ascontiguousarray(Wq[:, cs].astype(_BF)),
                    "wk": np.ascontiguousarray(Wk[:, cs].astype(_BF)),
                    "wv": np.ascontiguousarray(Wv[:, cs].astype(_BF)),
                    "bq": np.ascontiguousarray(bq[cs].reshape(DL, 1)),
                    "bk": np.ascontiguousarray(bk[cs].reshape(DL, 1)),
                    "bv": np.ascontiguousarray(bv[cs].reshape(DL, 1)),
                    "wo": np.ascontiguousarray(Wo[cs, :]),
                }
            )
    return in_maps


def kernel(query, key, value, Wq, bq, Wk, bk, Wv, bv, Wo, bo):
    global LAST_RESULTS
    query = np.asarray(query, dtype=np.float32)
    key = np.asarray(key, dtype=np.float32)
    value = np.asarray(value, dtype=np.float32)
    Wq = np.asarray(Wq, dtype=np.float32)
    Wk = np.asarray(Wk, dtype=np.float32)
    Wv = np.asarray(Wv, dtype=np.float32)
    Wo = np.asarray(Wo, dtype=np.float32)
    bq = np.asarray(bq, dtype=np.float32)
    bk = np.asarray(bk, dtype=np.float32)
    bv = np.asarray(bv, dtype=np.float32)
    bo = np.asarray(bo, dtype=np.float32)

    nc = _build()
    in_maps = _make_in_maps(query, key, value, Wq, bq, Wk, bk, Wv, bv, Wo)

    res = run_bass_kernel_spmd(nc, in_maps, list(range(B * HP)), trace=TRACE)
    LAST_RESULTS = res

    out = np.empty((B, S, D), dtype=np.float32)
    for b in range(B):
        acc = res.results[b * HP]["outT"].astype(np.float32)
        for hp in range(1, HP):
            acc = acc + res.results[b * HP + hp]["outT"]
        out[b] = acc.T + bo
    return out

